# revision 1
# baseline (speedup 1.0000x reference)
"""Trainium2 Bass kernel for nn_FChCombxValEncoder (HDC n-gram encoder).

Computation: idx = quantize(x) -> signal = signals_weight[idx] -> bind with
feat_weight -> 4-gram product with per-step D-rolls -> bundle sum -> sign.

Distribution: feature axis (4096) sharded across 8 cores (512 n-gram starts
each); per-core bundle partials are ReduceScattered so core m signs D-slice
[1250m, 1250m+1250); the host concatenates the 8 slices and applies the
final roll-by-3 (a pure layout permutation).

Layout: each partition p holds FOUR consecutive rows t = 4p+j as streams in
the free dim.  The full gathered signal rows live resident in SBUF (one
gather per stream in 3 column-waves, ~12 indirect DMAs total); feat streams
per D-chunk of 2000.  Per chunk:
    bind   fe *= sf[chunk cols]          (in-place, one 3D-AP DVE op)
    U_j  = S_j . S_{j+1}(+1)             j=0..2 one op; j=3 uses A' =
                                         S_0[p+1] via split SBUF->SBUF DMA
    Q_j  = U_j . U_{j+2}(+2)             j=0,1 one op; j=2,3 use V =
                                         U_{0,1}[p+1](+2) via SBUF->SBUF DMA
    R    = ones^T @ Q                    PSUM accumulate, 500-col segs
The 3 n-grams per core needing rows past the 512-row pack ("orphans",
starts 509..511) are computed ONCE at full width in a 20-partition overlap
layout (partition 20t+p holds row 509+t, cols [500p, 500p+504)), so the
three DVE products cost ~400ns each instead of ~3us per chunk; their
per-(chunk,seg) sums are folded into the same PSUM accumulators with K=1
matmuls.

All values are +/-1 so bf16 is exact; bundle partials are integers < 2^24 so
fp32 PSUM/ReduceScatter is exact; the output sign never sees zero (4093 odd
terms).  Index quantization is bit-exact via a host fp32 threshold table:
idx = #{k : x >= b_k}.
"""
import sys

sys.path.insert(0, "/opt/trn_rl_repo")

import numpy as np
import ml_dtypes

import concourse.bass as bass
import concourse.bacc as bacc
import concourse.tile as tile
import concourse.mybir as mybir
from concourse.bass_utils import run_bass_kernel_spmd

# ---- problem constants ----
MAX_VAL = 52000.0
MIN_VAL = -53000.0
NUM_LEVELS = 1000
NGRAM = 4
D = 10000
NFEAT = 4096
NCORE = 8

PER_CORE = 512
ROLL = NGRAM - 1

W = 2000                      # D-chunk width
NCHUNK = D // W               # 5
PS = 2005                     # feat stream pitch (chunk window width)
UP = 2004                     # U stream pitch (width WU=2002)
QP = 2000                     # Q stream pitch
WU = W + 2                    # 2002
SEG = 500                     # orphan flatten granularity (d = 500p + f)
SEGS = [(0, 512), (512, 1024), (1024, 1536), (1536, 2000)]  # PSUM-bank segs
SP = 10008                    # resident signal stream pitch / table row width
NORPH = 6                     # orphan rows per core (t = 509..514)
OPN = 20                      # orphan partitions per row
OW = 500                      # orphan cols per partition (d = 500p + f)
OWP = 504                     # orphan gathered width (OW + 4 halo/pad)
SLICE = D // NCORE            # 1250 output cols per core

F32 = mybir.dt.float32
BF16 = mybir.dt.bfloat16
I32 = mybir.dt.int32
_BF = ml_dtypes.bfloat16

NTH = NUM_LEVELS - 1


# ---------------------------------------------------------------- host prep
def _f2o(u):
    b = u.view(np.uint32).astype(np.int64)
    return np.where(b < 0x80000000, b + 0x80000000, 0xFFFFFFFF - b)


def _o2f(o):
    b = np.where(o >= 0x80000000, o - 0x80000000, 0xFFFFFFFF - o).astype(np.uint64)
    return b.astype(np.uint32).view(np.float32)


def _g(v):
    v = v.astype(np.float32)
    t = (v - np.float32(MIN_VAL)).astype(np.float32)
    t = (t / np.float32(MAX_VAL - MIN_VAL)).astype(np.float32)
    t = (t * np.float32(NUM_LEVELS - 1)).astype(np.float32)
    return np.clip(np.round(t), 0.0, float(NUM_LEVELS - 1))


def _thresholds():
    ks = np.arange(1, NUM_LEVELS, dtype=np.float32)
    lo = _f2o(np.full(ks.shape, np.float32(MIN_VAL) - np.float32(2.0)))
    hi = _f2o(np.full(ks.shape, np.float32(MAX_VAL) + np.float32(2.0)))
    for _ in range(64):
        mid = (lo + hi) // 2
        ge = _g(_o2f(mid)) >= ks
        hi = np.where(ge, mid, hi)
        lo = np.where(ge, lo, mid)
        if np.all(hi - lo <= 1):
            break
    return _o2f(hi)


_CACHE = {}


def _host_constants():
    if "thr" not in _CACHE:
        _CACHE["thr"] = np.tile(_thresholds()[None, :], (128, 1)).astype(np.float32)
        _CACHE["onr"] = np.ones((128, 1), dtype=_BF)
        _CACHE["onrf"] = np.ones((NCORE, 1), dtype=np.float32)
        sel = np.zeros((3 * OPN, OPN), dtype=_BF)
        for t in range(3):
            for p in range(OPN):
                sel[OPN * t + p, p] = 1.0
        _CACHE["sel"] = sel
        _CACHE["iotap"] = np.tile(np.arange(OPN, dtype=np.float32),
                                  NORPH).reshape(NORPH * OPN, 1)
        _CACHE["zrow"] = np.zeros((1, 2 * W), dtype=_BF)
        sh1 = np.zeros((128, 128), dtype=_BF)
        for i in range(127):
            sh1[i + 1, i] = 1.0          # V[m] = U[m+1]; col 127 zero
        _CACHE["sh1"] = sh1
    return _CACHE


# ---------------------------------------------------------------- program
def _build_program():
    nc = bacc.Bacc("TRN2", target_bir_lowering=False, debug=False,
                   num_devices=NCORE)

    x4_d = nc.dram_tensor("x4", (128, 4), F32, kind="ExternalInput")
    xo_d = nc.dram_tensor("xo3", (NORPH * OPN, 1), F32, kind="ExternalInput")
    iot_d = nc.dram_tensor("iotap", (NORPH * OPN, 1), F32, kind="ExternalInput")
    thr_d = nc.dram_tensor("thr", (128, NTH), F32, kind="ExternalInput")
    table_d = nc.dram_tensor("table", (NUM_LEVELS, SP), BF16, kind="ExternalInput")
    tabo_d = nc.dram_tensor("table_o", (NUM_LEVELS * OPN, OWP), BF16,
                            kind="ExternalInput")
    feat4_d = nc.dram_tensor("feat4", (NCHUNK, 128, 4 * PS), BF16,
                             kind="ExternalInput")
    feato_d = nc.dram_tensor("feat_o", (NORPH * OPN, OWP), BF16,
                             kind="ExternalInput")
    sel_d = nc.dram_tensor("sel", (3 * OPN, OPN), BF16, kind="ExternalInput")
    onr_d = nc.dram_tensor("ones_red", (128, 1), BF16, kind="ExternalInput")
    onrf_d = nc.dram_tensor("ones_f32", (NCORE, 1), F32, kind="ExternalInput")
    sh1_d = nc.dram_tensor("sh1", (128, 128), BF16, kind="ExternalInput")
    zrow_d = nc.dram_tensor("zrow", (1, 2 * W), BF16, kind="ExternalInput")
    out_d = nc.dram_tensor("out", (1, SLICE), F32, kind="ExternalOutput")

    cc_in = nc.dram_tensor("cc_in", (1, D), F32)
    cc_a2a = nc.dram_tensor("cc_a2a", (1, D), F32)
    orphf_d = nc.dram_tensor("orphf_d", (1, D), BF16)

    # raw tensors (manual double-buffer; boundary row kept zero)
    a1_raw = [nc.alloc_sbuf_tensor(f"a1f{i}", [128, WU], BF16).ap()
              for i in range(2)]

    # gather column pieces [start, width): piece 0 upfront (chunk 0),
    # piece c+1 issued during chunk c; <= 2005 cols keeps each gathered
    # row a single DMA descriptor
    PIECES = [(0, PS), (PS, 2000), (PS + 2000, 2000), (PS + 4000, 2000),
              (PS + 6000, SP - PS - 6000)]

    with tile.TileContext(nc) as tc:
        with tc.tile_pool(name="const", bufs=1) as cpool, \
             tc.tile_pool(name="loads", bufs=2) as lpool, \
             tc.tile_pool(name="work", bufs=1) as wpool, \
             tc.tile_pool(name="orph", bufs=1) as opool, \
             tc.tile_pool(name="pacc", bufs=1, space="PSUM") as pacc, \
             tc.tile_pool(name="pseg", bufs=2, space="PSUM") as pseg, \
             tc.tile_pool(name="porp", bufs=1, space="PSUM") as porp:

            sf = cpool.tile([128, 4 * SP], BF16, tag="sf")
            thr = cpool.tile([128, NTH], F32)
            nc.sync.dma_start(out=thr[:, :], in_=thr_d[:, :])
            x4 = cpool.tile([128, 4], F32)
            nc.sync.dma_start(out=x4[:, :], in_=x4_d[:, :])
            xo = cpool.tile([NORPH * OPN, 1], F32)
            nc.sync.dma_start(out=xo[:, :], in_=xo_d[:, :])
            iot = cpool.tile([NORPH * OPN, 1], F32)
            nc.sync.dma_start(out=iot[:, :], in_=iot_d[:, :])
            onr = cpool.tile([128, 1], BF16)
            nc.sync.dma_start(out=onr[:, :], in_=onr_d[:, :])
            onrf = cpool.tile([NCORE, 1], F32)
            nc.sync.dma_start(out=onrf[:, :], in_=onrf_d[:, :])
            sel = cpool.tile([3 * OPN, OPN], BF16)
            nc.scalar.dma_start(out=sel[:, :], in_=sel_d[:, :])
            sh1 = cpool.tile([128, 128], BF16)
            nc.scalar.dma_start(out=sh1[:, :], in_=sh1_d[:, :])
            fe_o = opool.tile([NORPH * OPN, OWP], BF16, tag="fe_o")
            nc.scalar.dma_start(out=fe_o[:, :], in_=feato_d[:, :])

            # zero boundary row of the raw double-buffers (once)
            for a1 in a1_raw:
                nc.sync.dma_start(out=a1[127:128, :], in_=zrow_d[0:1, 0:WU])

            # ---- per-stream indices (is_le count against threshold LUT) ----
            idx_tiles = []
            for j in range(4):
                ge = opool.tile([128, NTH], BF16, tag="ge")
                nc.vector.tensor_scalar(
                    out=ge[:, :], in0=thr[:, :], scalar1=x4[:, j:j + 1],
                    scalar2=None, op0=mybir.AluOpType.is_le)
                idxf = opool.tile([128, 1], F32, tag=f"idxf{j}")
                nc.vector.tensor_reduce(out=idxf[:, :], in_=ge[:, :],
                                        axis=mybir.AxisListType.X,
                                        op=mybir.AluOpType.add)
                it = cpool.tile([128, 1], I32, tag=f"idx{j}")
                nc.vector.tensor_copy(out=it[:, :], in_=idxf[:, :])
                idx_tiles.append(it)
            # orphan composite index: idx*20 + p
            geo = opool.tile([NORPH * OPN, NTH], BF16, tag="ge")
            nc.vector.tensor_scalar(
                out=geo[:, :], in0=thr[0:NORPH * OPN, :], scalar1=xo[:, 0:1],
                scalar2=None, op0=mybir.AluOpType.is_le)
            idxfo = opool.tile([NORPH * OPN, 1], F32, tag="idxfo")
            nc.vector.tensor_reduce(out=idxfo[:, :], in_=geo[:, :],
                                    axis=mybir.AxisListType.X,
                                    op=mybir.AluOpType.add)
            idx2f = opool.tile([NORPH * OPN, 1], F32, tag="idx2f")
            nc.vector.tensor_scalar(
                out=idx2f[:, :], in0=idxfo[:, :], scalar1=float(OPN),
                scalar2=iot[:, 0:1], op0=mybir.AluOpType.mult,
                op1=mybir.AluOpType.add)
            idxo = cpool.tile([NORPH * OPN, 1], I32, tag="idxo")
            nc.vector.tensor_copy(out=idxo[:, :], in_=idx2f[:, :])

            # ---- upfront gathers: chunk-0 piece + orphan rows ----
            def gather_piece(pc):
                c0, w0 = PIECES[pc]
                for j in range(4):
                    nc.gpsimd.indirect_dma_start(
                        out=sf[:, j * SP + c0:j * SP + c0 + w0],
                        out_offset=None,
                        in_=table_d[:, :],
                        in_offset=bass.IndirectOffsetOnAxis(
                            ap=idx_tiles[j][:, 0:1], axis=0),
                        element_offset=c0,
                    )

            gather_piece(0)
            s_o = opool.tile([NORPH * OPN, OWP], BF16, tag="s_o")
            nc.gpsimd.indirect_dma_start(
                out=s_o[:, :], out_offset=None,
                in_=tabo_d[:, :],
                in_offset=bass.IndirectOffsetOnAxis(ap=idxo[:, 0:1], axis=0),
                element_offset=0,
            )

            # ---- orphan compute (once, full width) ----
            nc.vector.tensor_tensor(out=s_o[:, :], in0=s_o[:, :],
                                    in1=fe_o[:, :], op=mybir.AluOpType.mult)
            s_o1 = opool.tile([5 * OPN, OWP - 1], BF16, tag="s_o1")
            nc.scalar.dma_start(out=s_o1[:, :], in_=s_o[OPN:6 * OPN, 1:OWP])
            u_o = opool.tile([5 * OPN, OWP - 1], BF16, tag="u_o")
            nc.vector.tensor_tensor(out=u_o[:, :],
                                    in0=s_o[0:5 * OPN, 0:OWP - 1],
                                    in1=s_o1[:, :],
                                    op=mybir.AluOpType.mult)
            u_o2 = opool.tile([3 * OPN, OWP - 3], BF16, tag="u_o2")
            nc.scalar.dma_start(out=u_o2[:, :],
                                in_=u_o[2 * OPN:5 * OPN, 2:OWP - 1])
            q_o = opool.tile([3 * OPN, OWP - 3], BF16, tag="q_o")
            nc.vector.tensor_tensor(out=q_o[:, :],
                                    in0=u_o[0:3 * OPN, 0:OWP - 3],
                                    in1=u_o2[:, :],
                                    op=mybir.AluOpType.mult)
            psum_o = porp.tile([OPN, OW], F32, tag="psum_o")
            nc.tensor.matmul(out=psum_o[:, :], lhsT=sel[:, :],
                             rhs=q_o[:, 0:OW], start=True, stop=True)
            orph = opool.tile([OPN, OW], BF16, tag="orph")
            nc.scalar.copy(out=orph[:, :], in_=psum_o[:, :])
            # flatten via DRAM so per-chunk matmul rhs slices sit at
            # partition 0 without reserving a 20KB SBUF column
            nc.scalar.dma_start(out=orphf_d[0:1, :], in_=orph[:, :])

            # ---- main chunk loop ----
            sf_r = sf[:, :].rearrange("p (s w) -> p s w", s=4)
            for c in range(NCHUNK):
                e0 = c * W

                fe = lpool.tile([128, 4 * PS], BF16, tag="fe")
                nc.sync.dma_start(out=fe[:, :], in_=feat4_d[c, :, :])
                fe_r = fe[:, :].rearrange("p (s w) -> p s w", s=4)
                orc = lpool.tile([1, W], BF16, tag="orc")
                nc.scalar.dma_start(out=orc[:, :], in_=orphf_d[0:1, e0:e0 + W])

                # prefetch next chunk's signal columns (Q7 is idle here)
                if c + 1 < NCHUNK:
                    gather_piece(c + 1)

                # bind in place: fe = sf[chunk] * fe
                nc.vector.tensor_tensor(
                    out=fe_r[:, :, :],
                    in0=fe_r[:, :, :],
                    in1=sf_r[:, :, e0:e0 + PS],
                    op=mybir.AluOpType.mult)

                # A'[p] = S_0[p+1, 1:2003]  (split SBUF->SBUF DMA on the
                # gpsimd queue so the sync/scalar FIFOs never head-of-line
                # block prefetches behind this bind-dependent copy)
                a1 = a1_raw[c % 2]
                for k in range(8):
                    n = 16 if k < 7 else 15
                    nc.gpsimd.dma_start(
                        out=a1[16 * k:16 * k + n, :],
                        in_=fe[16 * k + 1:16 * k + 1 + n, 1:1 + WU])

                # U
                u4 = wpool.tile([128, 4 * UP], BF16, tag="u4")
                u4_r = u4[:, :].rearrange("p (s w) -> p s w", s=4)
                nc.vector.tensor_tensor(
                    out=u4_r[:, 0:3, 0:WU],
                    in0=fe_r[:, 0:3, 0:WU],
                    in1=fe_r[:, 1:4, 1:1 + WU],
                    op=mybir.AluOpType.mult)
                nc.vector.tensor_tensor(
                    out=u4[:, 3 * UP:3 * UP + WU],
                    in0=fe[:, 3 * PS:3 * PS + WU],
                    in1=a1[:, :],
                    op=mybir.AluOpType.mult)

                # V[p, s, d] = U_s[p+1, d+2] via shift matmul (sh1 col 127
                # is zero so V row 127 = 0)
                v4 = wpool.tile([128, 2 * W], BF16, tag="v4")
                v4_r = v4[:, :].rearrange("p (s w) -> p s w", s=2)
                for s in range(2):
                    for a0, a1s in SEGS:
                        vp = pseg.tile([128, 512], F32, tag="v")
                        nc.tensor.matmul(
                            out=vp[:, 0:a1s - a0],
                            lhsT=sh1[:, :],
                            rhs=u4[:, s * UP + 2 + a0:s * UP + 2 + a1s],
                            start=True, stop=True)
                        nc.scalar.copy(out=v4[:, s * W + a0:s * W + a1s],
                                       in_=vp[:, 0:a1s - a0])

                # Q
                q4 = wpool.tile([128, 4 * QP], BF16, tag="q4")
                q4_r = q4[:, :].rearrange("p (s w) -> p s w", s=4)
                nc.vector.tensor_tensor(
                    out=q4_r[:, 0:2, :],
                    in0=u4_r[:, 0:2, 0:W],
                    in1=u4_r[:, 2:4, 2:2 + W],
                    op=mybir.AluOpType.mult)
                nc.vector.tensor_tensor(
                    out=q4_r[:, 2:4, :],
                    in0=u4_r[:, 2:4, 0:W],
                    in1=v4_r[:, :, :],
                    op=mybir.AluOpType.mult)

                # bundle reduce: PSUM-bank-aligned segs x (4 streams + orphan)
                accp = pacc.tile([1, W], F32, tag="acc")
                for a0, a1s in SEGS:
                    for j in range(4):
                        nc.tensor.matmul(out=accp[0:1, a0:a1s],
                                         lhsT=onr[:, 0:1],
                                         rhs=q4[:, j * QP + a0:j * QP + a1s],
                                         start=(j == 0), stop=False)
                    nc.tensor.matmul(out=accp[0:1, a0:a1s],
                                     lhsT=onr[0:1, 0:1],
                                     rhs=orc[0:1, a0:a1s],
                                     start=False, stop=True)
                stg = wpool.tile([1, W], F32, tag="stg")
                nc.scalar.copy(out=stg[:, :], in_=accp[0:1, :])
                nc.scalar.dma_start(out=cc_in[0:1, e0:e0 + W], in_=stg[:, :])

            # ---- cross-core exchange: core m receives every core's slice
            # m (one point-to-point hop), then sums the 8 rows on the PE ----
            a2a_view = lambda t: t[:, :].rearrange("o (r s) -> (o r) s",
                                                   r=NCORE)
            nc.gpsimd.collective_compute(
                "AllToAll", mybir.AluOpType.bypass,
                ins=[a2a_view(cc_in)],
                outs=[a2a_view(cc_a2a)],
                replica_groups=[list(range(NCORE))],
            )
            g8 = wpool.tile([NCORE, SLICE], F32, tag="g8")
            nc.sync.dma_start(out=g8[:, :], in_=a2a_view(cc_a2a))
            accf = pacc.tile([1, W], F32, tag="acc")
            for a0, a1s in ((0, 512), (512, 1024), (1024, SLICE)):
                nc.tensor.matmul(out=accf[0:1, a0:a1s],
                                 lhsT=onrf[:, 0:1],
                                 rhs=g8[:, a0:a1s], start=True, stop=True)
            t1 = wpool.tile([1, SLICE], F32, tag="fin2")
            nc.vector.tensor_scalar(out=t1[:, :], in0=accf[0:1, 0:SLICE],
                                    scalar1=0.0, scalar2=2.0,
                                    op0=mybir.AluOpType.is_gt,
                                    op1=mybir.AluOpType.mult)
            sg = wpool.tile([1, SLICE], F32, tag="fin3")
            nc.vector.tensor_scalar(out=sg[:, :], in0=t1[:, :], scalar1=-1.0,
                                    scalar2=None, op0=mybir.AluOpType.add)
            nc.sync.dma_start(out=out_d[0:1, :], in_=sg[:, :])

    nc.compile()
    return nc


TRACE = False
LAST_RESULT = None


def _pad_rows(fw, base, n):
    """rows [base, base+n) of fw, zero-padded past NFEAT, with SP wrap."""
    out = np.zeros((n, SP), dtype=_BF)
    nreal = max(0, min(n, NFEAT - base))
    if nreal > 0:
        fb = fw[base:base + nreal].astype(_BF)
        out[:nreal, :D] = fb
        out[:nreal, D:] = fb[:, :SP - D]
    return out


def _overlap_slices(rows):
    """(n, SP) wrapped rows -> (n*OPN, OWP): row i*OPN+p = row i cols
    [500p, 500p+OWP)."""
    n = rows.shape[0]
    out = np.zeros((n * OPN, OWP), dtype=_BF)
    for p in range(OPN):
        out[p::OPN] = rows[:, OW * p:OW * p + OWP]
    return out


def _make_in_maps(xf, sw, fw, consts):
    table = np.empty((NUM_LEVELS, SP), dtype=_BF)
    table[:, :D] = sw.astype(_BF)
    table[:, D:] = table[:, :SP - D]

    # orphan table: row r*OPN + p = table[r, 500p : 500p+OWP]
    table_o = _overlap_slices(table)

    in_maps = []
    for m in range(NCORE):
        base = PER_CORE * m

        # packed feat: feat4[c, p, j*PS + e] = fw_pad[base + 4p + j, c*W + e]
        fp = _pad_rows(fw, base, PER_CORE)              # (512, SP)
        fp4 = np.zeros((NCHUNK, 128, 4 * PS), dtype=_BF)
        for c in range(NCHUNK):
            sl = fp[:, c * W:c * W + PS]                # (512, PS)
            fp4[c] = sl.reshape(128, 4 * PS)

        # orphan feat rows base+509 .. base+514 in overlap layout
        fo = _pad_rows(fw, base + PER_CORE - 3, NORPH)  # (6, SP)
        fo2 = _overlap_slices(fo)                       # (120, OWP)

        xr = np.full(PER_CORE + NORPH, xf[-1], dtype=np.float32)
        nreal = min(PER_CORE + 3, NFEAT - base)
        xr[:nreal] = xf[base:base + nreal]
        x4 = xr[:PER_CORE].reshape(128, 4).copy()
        xo3 = np.repeat(xr[PER_CORE - 3:PER_CORE - 3 + NORPH],
                        OPN).reshape(NORPH * OPN, 1).copy()

        in_maps.append({
            "x4": x4,
            "xo3": xo3,
            "iotap": consts["iotap"],
            "thr": consts["thr"],
            "table": table,
            "table_o": table_o,
            "feat4": fp4,
            "feat_o": fo2,
            "sel": consts["sel"],
            "ones_red": consts["onr"],
            "ones_f32": consts["onrf"],
            "sh1": consts["sh1"],
            "zrow": consts["zrow"],
        })
    return in_maps


def kernel(x, signals_weight, feat_weight):
    global LAST_RESULT
    consts = _host_constants()

    if "nc" not in _CACHE:
        _CACHE["nc"] = _build_program()
    nc = _CACHE["nc"]

    xf = np.asarray(x, dtype=np.float32).reshape(-1)
    sw = np.asarray(signals_weight, dtype=np.float32)
    fw = np.asarray(feat_weight, dtype=np.float32)
    in_maps = _make_in_maps(xf, sw, fw, consts)

    res = run_bass_kernel_spmd(nc, in_maps, list(range(NCORE)), trace=TRACE)
    LAST_RESULT = res
    full = np.concatenate(
        [np.asarray(res.results[m]["out"], dtype=np.float32).reshape(-1)
         for m in range(NCORE)])
    return np.roll(full, ROLL)[None, :]



# revision 18
# speedup vs baseline: 1.1152x; 1.1152x over previous
"""Trainium2 Bass kernel for nn_FChCombxValEncoder (HDC n-gram encoder).

Computation: idx = quantize(x) -> signal = signals_weight[idx] -> bind with
feat_weight -> 4-gram product with per-step D-rolls -> bundle sum -> sign.

Distribution (v2, D-shard): the hypervector dimension D=10000 is sharded
across the 8 cores -- core m owns output columns [1250m, 1250m+1250).  Each
core sees ALL 4096 feature rows but only a 1255-column slice (with mod-D
wrap) of the level table and feat weights, so the whole n-gram bundle sum
for its slice is local: NO collective, no orphan rows, no cross-core
reduce.  The host concatenates the 8 slices and applies the final
roll-by-3 (a pure layout permutation).

Layout: partition p holds rows 32p..32p+31 as 32 streams of width TW=1256
in the free dim.  Row shifts i->i+1 are stream shifts (intra-partition)
except stream 31 -> next partition's stream 0, handled by two small
partition-shift SBUF->SBUF DMAs (A = S_0[p+1], U' = U_{0,1}[p+1]) whose
boundary row 127 is memset to zero -- which also zeroes the 3 invalid
n-gram starts 4093..4095 automatically.

Pipeline: 8 groups of 4 streams. Per group: feat DMA + signal gather ->
bind (S = sig*feat, in place over feat) -> U_s = S_s . S_{s+1}(+1) ->
Q_s = U_s . U_{s+2}(+2) -> T_t = Q_t + Q_{t+16} -> PSUM-accumulated
ones-matmul over partitions (3 segs x 16 streams).  U overwrites the dead
gathered-signal buffer, Q overwrites dead S, T overwrites dead U, so peak
SBUF is ~2 x 80KB/partition.

Index quantization is bit-exact via a host-built bucket LUT: b =
trunc/round((x-MIN)*NB/RANGE) (any rounding within +-1 bucket is fine by
construction), then idx = base[b] + (x >= t[b]) where each bucket's
extended window provably contains at most one of the 999 exact fp32
thresholds (bisected so that #{thr <= x} == reference idx).

All values are +/-1 so bf16 is exact; bundle partials are integers < 2^12
so fp32 PSUM is exact; the output sign never sees zero (4093 odd terms).
"""
import sys

sys.path.insert(0, "/opt/trn_rl_repo")

import numpy as np
import ml_dtypes

import concourse.bass as bass
import concourse.bacc as bacc
import concourse.tile as tile
import concourse.mybir as mybir
from concourse.bass_utils import run_bass_kernel_spmd
from concourse import library_config

# ---- problem constants ----
MAX_VAL = 52000.0
MIN_VAL = -53000.0
RANGE = MAX_VAL - MIN_VAL
NUM_LEVELS = 1000
NGRAM = 4
D = 10000
NFEAT = 4096
NCORE = 8

ROLL = NGRAM - 1
SLICE = D // NCORE            # 1250 output cols per core

NS = 32                       # streams (rows) per partition
TW = 1280                     # stream pitch (2560B, dma_gather 256B-multiple)
NG = 8                        # pipeline groups
GS = NS // NG                 # 4 streams per group

NB = 4096                     # quantizer buckets
NBP = NB + 4                  # padded bucket table rows
BSCALE = float(np.float32(NB / RANGE))

SEGS = [(0, 512), (512, 1024), (1024, 1252)]   # PSUM-bank matmul segs

F32 = mybir.dt.float32
BF16 = mybir.dt.bfloat16
I32 = mybir.dt.int32
I16 = mybir.dt.int16
_BF = ml_dtypes.bfloat16

NTH = NUM_LEVELS - 1


# ---------------------------------------------------------------- host prep
def _f2o(u):
    b = u.view(np.uint32).astype(np.int64)
    return np.where(b < 0x80000000, b + 0x80000000, 0xFFFFFFFF - b)


def _o2f(o):
    b = np.where(o >= 0x80000000, o - 0x80000000, 0xFFFFFFFF - o).astype(np.uint64)
    return b.astype(np.uint32).view(np.float32)


def _g(v):
    v = v.astype(np.float32)
    t = (v - np.float32(MIN_VAL)).astype(np.float32)
    t = (t / np.float32(MAX_VAL - MIN_VAL)).astype(np.float32)
    t = (t * np.float32(NUM_LEVELS - 1)).astype(np.float32)
    return np.clip(np.round(t), 0.0, float(NUM_LEVELS - 1))


def _thresholds():
    ks = np.arange(1, NUM_LEVELS, dtype=np.float32)
    lo = _f2o(np.full(ks.shape, np.float32(MIN_VAL) - np.float32(2.0)))
    hi = _f2o(np.full(ks.shape, np.float32(MAX_VAL) + np.float32(2.0)))
    for _ in range(64):
        mid = (lo + hi) // 2
        ge = _g(_o2f(mid)) >= ks
        hi = np.where(ge, mid, hi)
        lo = np.where(ge, lo, mid)
        if np.all(hi - lo <= 1):
            break
    return _o2f(hi)


def _bucket_table():
    """(NBP,) t and (NBP,) base f32 tables: idx(x) = base[b] + (x >= t[b])
    for any device bucket b within +-1.02 of (x-MIN)*NB/RANGE."""
    thr = _thresholds().astype(np.float64)          # (999,) sorted
    w = RANGE / NB
    t = np.full(NBP, 3.0e38, dtype=np.float32)
    base = np.zeros(NBP, dtype=np.float32)
    bs = np.arange(NBP, dtype=np.float64)
    lo = MIN_VAL + (bs - 1.02) * w
    hi = MIN_VAL + (bs + 1.02) * w
    for b in range(NBP):
        inb = np.nonzero((thr > lo[b]) & (thr <= hi[b]))[0]
        assert len(inb) <= 1, f"bucket {b} holds {len(inb)} thresholds"
        base[b] = np.count_nonzero(thr <= lo[b])
        if len(inb):
            t[b] = np.float32(thr[inb[0]])
    w64 = np.zeros((NBP, 64), dtype=np.float32)
    w64[:, 0] = t
    w64[:, 1] = base
    return w64


_CACHE = {}


def _host_constants():
    if "thr" not in _CACHE:
        _CACHE["thr"] = np.tile(_thresholds()[None, :], (128, 1)).astype(np.float32)
        _CACHE["zrow"] = np.zeros((1, 2 * TW), dtype=_BF)
    return _CACHE


# ---------------------------------------------------------------- program
def _build_program():
    nc = bacc.Bacc("TRN2", target_bir_lowering=False, debug=False,
                   num_devices=NCORE)

    x32_d = nc.dram_tensor("x32", (128, NS), F32, kind="ExternalInput")
    thr_d = nc.dram_tensor("thr", (128, NTH), F32, kind="ExternalInput")
    table_d = nc.dram_tensor("table", (NUM_LEVELS, TW), BF16,
                             kind="ExternalInput")
    feat_d = nc.dram_tensor("feat", (NG, 128, GS * TW), BF16,
                            kind="ExternalInput")
    zrow_d = nc.dram_tensor("zrow", (1, 2 * TW), BF16, kind="ExternalInput")
    out_d = nc.dram_tensor("out", (1, SLICE), F32, kind="ExternalOutput")
    if DEBUG:
        dbg_idx_d = nc.dram_tensor("dbg_idx", (128, NS), I32,
                                   kind="ExternalOutput")
        dbg_sig_d = nc.dram_tensor("dbg_sig", (128, TW), BF16,
                                   kind="ExternalOutput")
        dbg_s_d = nc.dram_tensor("dbg_s", (128, TW), BF16,
                                 kind="ExternalOutput")
        dbg_acc_d = nc.dram_tensor("dbg_acc", (1, 1252), F32,
                                   kind="ExternalOutput")

    # raw tensors for partition-shifted copies (row 127 stays zero)
    a_raw = nc.alloc_sbuf_tensor("a_shift", [128, TW], BF16).ap()
    up_raw = nc.alloc_sbuf_tensor("up_shift", [128, 2 * TW], BF16).ap()

    with tile.TileContext(nc) as tc:
        with tc.tile_pool(name="const", bufs=1) as cpool, \
             tc.tile_pool(name="work", bufs=1) as wpool, \
             tc.tile_pool(name="pacc", bufs=1, space="PSUM") as pacc:

            # ---- constants / index computation ----
            onr = cpool.tile([128, 1], BF16, tag="onr")
            nc.vector.memset(onr[:, :], 1.0)
            nc.scalar.dma_start(out=a_raw[127:128, :], in_=zrow_d[0:1, 0:TW])
            nc.scalar.dma_start(out=up_raw[127:128, :], in_=zrow_d[0:1, :])

            x32 = cpool.tile([128, NS], F32, tag="x32")
            nc.sync.dma_start(out=x32[:, :], in_=x32_d[:, :])
            thr = cpool.tile([128, NTH], F32, tag="thr")
            nc.sync.dma_start(out=thr[:, :], in_=thr_d[:, :])

            # idx[p, s] = #{thr <= x[p, s]} via is_le compare with fused
            # free-dim accumulate; computed one pipeline group ahead
            ge = cpool.tile([128, NTH], BF16, tag="ge")
            idxf = cpool.tile([128, NS], F32, tag="idxf")
            idxn = cpool.tile([128, NS], I32, tag="idxn")

            def idx_group(g):
                for j in range(GS):
                    s = g * GS + j
                    nc.vector.tensor_scalar(
                        out=ge[:, :], in0=thr[:, :],
                        scalar1=x32[:, s:s + 1], scalar2=0.0,
                        op0=mybir.AluOpType.is_le,
                        op1=mybir.AluOpType.add,
                        accum_out=idxf[:, s:s + 1])
                nc.vector.tensor_copy(out=idxn[:, g * GS:(g + 1) * GS],
                                      in_=idxf[:, g * GS:(g + 1) * GS])

            idx_group(0)
            if DEBUG:
                nc.sync.dma_start(out=dbg_idx_d[:, :], in_=idxn[:, :])

            # ---- main buffers ----
            sb = wpool.tile([128, NS * TW], BF16, tag="sb")    # feat -> S -> Q
            gb = wpool.tile([128, NS * TW], BF16, tag="gb")    # sig -> U -> T
            sb_r = sb[:, :].rearrange("p (s w) -> p s w", s=NS)
            gb_r = gb[:, :].rearrange("p (s w) -> p s w", s=NS)

            acc = pacc.tile([1, 1252], F32, tag="acc")

            def u_window(lo, hi):
                """U_s = S_s * S_{s+1}[+1] for s in [lo, hi) (intra-partition)."""
                nc.vector.tensor_tensor(
                    out=gb_r[:, lo:hi, 0:1254],
                    in0=sb_r[:, lo:hi, 0:1254],
                    in1=sb_r[:, lo + 1:hi + 1, 1:1255],
                    op=mybir.AluOpType.mult)

            def q_window(lo, hi):
                """Q_s = U_s * U_{s+2}[+2] for s in [lo, hi) (intra-partition)."""
                nc.vector.tensor_tensor(
                    out=sb_r[:, lo:hi, 0:1252],
                    in0=gb_r[:, lo:hi, 0:1252],
                    in1=gb_r[:, lo + 2:hi + 2, 2:1254],
                    op=mybir.AluOpType.mult)

            def t_pair(t0, t1):
                """T_t = Q_t + Q_{t+16} for t in [t0, t1), into U slot t."""
                nc.vector.tensor_tensor(
                    out=gb_r[:, t0:t1, 0:1252],
                    in0=sb_r[:, t0:t1, 0:1252],
                    in1=sb_r[:, t0 + 16:t1 + 16, 0:1252],
                    op=mybir.AluOpType.add)

            def t_matmuls(t):
                for a0, a1 in SEGS:
                    nc.tensor.matmul(out=acc[0:1, a0:a1],
                                     lhsT=onr[:, 0:1],
                                     rhs=gb[:, t * TW + a0:t * TW + a1],
                                     start=(t == 0), stop=(t == 15))

            # ---- pipelined groups ----
            for g in range(NG):
                s0 = g * GS
                nc.sync.dma_start(out=sb[:, s0 * TW:(s0 + GS) * TW],
                                  in_=feat_d[g, :, :])
                for j in range(GS):
                    s = g * GS + j
                    nc.gpsimd.indirect_dma_start(
                        out=gb[:, s * TW:(s + 1) * TW], out_offset=None,
                        in_=table_d[:, :],
                        in_offset=bass.IndirectOffsetOnAxis(
                            ap=idxn[:, s:s + 1], axis=0),
                        element_offset=0)
                if g + 1 < NG:
                    idx_group(g + 1)
                if DEBUG and g == 0:
                    nc.sync.dma_start(out=dbg_sig_d[:, :], in_=gb[:, 0:TW])
                # bind S = sig * feat (in place over feat)
                nc.vector.tensor_tensor(
                    out=sb_r[:, s0:s0 + GS, :],
                    in0=sb_r[:, s0:s0 + GS, :],
                    in1=gb_r[:, s0:s0 + GS, :],
                    op=mybir.AluOpType.mult)
                if DEBUG and g == 0:
                    nc.sync.dma_start(out=dbg_s_d[:, :], in_=sb[:, 0:TW])

                if g == 0:
                    # A[p] = S_0[p+1] for U_31 (boundary row 127 is zero)
                    nc.scalar.dma_start(out=a_raw[0:127, :],
                                        in_=sb[1:128, 0:TW])
                    u_window(0, GS - 1)                      # U_0..2
                    # U'[p] = U_{0,1}[p+1] for Q_30,31
                    nc.scalar.dma_start(out=up_raw[0:127, :],
                                        in_=gb[1:128, 0:2 * TW])
                else:
                    u_window(s0 - 1, s0 + GS - 1)            # U_{4g-1}..{4g+2}
                    # Q streams [4(g-1) .. 4(g-1)+3] need U <= 4g+1 (done)
                    q0 = (g - 1) * GS
                    q_window(q0, q0 + GS)
                    if g >= 5:
                        # T_t = Q_t + Q_{t+16}: second half Q just arrived
                        t0 = (g - 5) * GS
                        t_pair(t0, t0 + GS)
                        for t in range(t0, t0 + GS):
                            t_matmuls(t)

            # ---- tail: boundary streams ----
            # U_31 = S_31 * A[+1]  (all 2D APs)
            nc.vector.tensor_tensor(
                out=gb[:, 31 * TW:31 * TW + 1254],
                in0=sb[:, 31 * TW:31 * TW + 1254],
                in1=a_raw[:, 1:1255],
                op=mybir.AluOpType.mult)
            # Q_28,29 (need U_30, U_31)
            q_window(28, 30)
            # Q_30 = U_30 * U'_0[+2];  Q_31 = U_31 * U'_1[+2]
            up_r = up_raw[:, :].rearrange("p (s w) -> p s w", s=2)
            nc.vector.tensor_tensor(
                out=sb_r[:, 30:32, 0:1252],
                in0=gb_r[:, 30:32, 0:1252],
                in1=up_r[:, 0:2, 2:1254],
                op=mybir.AluOpType.mult)
            t_pair(12, 16)
            for t in range(12, 16):
                t_matmuls(t)

            # ---- sign + output ----
            if DEBUG:
                dacc = wpool.tile([1, 1252], F32, tag="dacc")
                nc.scalar.copy(out=dacc[:, :], in_=acc[0:1, :])
                nc.sync.dma_start(out=dbg_acc_d[0:1, :], in_=dacc[:, :])
            t1 = wpool.tile([1, SLICE], F32, tag="fin2")
            nc.vector.tensor_scalar(out=t1[:, :], in0=acc[0:1, 0:SLICE],
                                    scalar1=0.0, scalar2=2.0,
                                    op0=mybir.AluOpType.is_gt,
                                    op1=mybir.AluOpType.mult)
            sg = wpool.tile([1, SLICE], F32, tag="fin3")
            nc.vector.tensor_scalar(out=sg[:, :], in0=t1[:, :], scalar1=-1.0,
                                    scalar2=None, op0=mybir.AluOpType.add)
            nc.sync.dma_start(out=out_d[0:1, :], in_=sg[:, :])

    nc.compile()
    return nc


TRACE = False
DEBUG = False
LAST_RESULT = None


def _make_in_maps(xf, sw, fw, consts):
    in_maps = []
    x32 = xf.reshape(128, NS).astype(np.float32)

    for m in range(NCORE):
        c0 = SLICE * m
        cols = (c0 + np.arange(TW)) % D
        table = sw[:, cols].astype(_BF)                       # (1000, TW)
        fwc = fw[:, cols].astype(_BF)                         # (4096, TW)
        feat = np.ascontiguousarray(
            fwc.reshape(128, NG, GS, TW)
               .transpose(1, 0, 2, 3)
               .reshape(NG, 128, GS * TW))
        in_maps.append({
            "x32": x32,
            "thr": consts["thr"],
            "table": table,
            "feat": feat,
            "zrow": consts["zrow"],
        })
    return in_maps


def kernel(x, signals_weight, feat_weight):
    global LAST_RESULT
    consts = _host_constants()

    if "nc" not in _CACHE:
        _CACHE["nc"] = _build_program()
    nc = _CACHE["nc"]

    xf = np.asarray(x, dtype=np.float32).reshape(-1)
    sw = np.asarray(signals_weight, dtype=np.float32)
    fw = np.asarray(feat_weight, dtype=np.float32)
    in_maps = _make_in_maps(xf, sw, fw, consts)

    res = run_bass_kernel_spmd(nc, in_maps, list(range(NCORE)), trace=TRACE)
    LAST_RESULT = res
    full = np.concatenate(
        [np.asarray(res.results[m]["out"], dtype=np.float32).reshape(-1)
         for m in range(NCORE)])
    return np.roll(full, ROLL)[None, :]


# revision 19
# speedup vs baseline: 1.1661x; 1.0456x over previous
"""Trainium2 Bass kernel for nn_FChCombxValEncoder (HDC n-gram encoder).

Computation: idx = quantize(x) -> signal = signals_weight[idx] -> bind with
feat_weight -> 4-gram product with per-step D-rolls -> bundle sum -> sign.

Distribution (v2, D-shard): the hypervector dimension D=10000 is sharded
across the 8 cores -- core m owns output columns [1250m, 1250m+1250).  Each
core sees ALL 4096 feature rows but only a 1255-column slice (with mod-D
wrap) of the level table and feat weights, so the whole n-gram bundle sum
for its slice is local: NO collective, no orphan rows, no cross-core
reduce.  The host concatenates the 8 slices and applies the final
roll-by-3 (a pure layout permutation).

Layout: partition p holds rows 32p..32p+31 as 32 streams of width TW=1256
in the free dim.  Row shifts i->i+1 are stream shifts (intra-partition)
except stream 31 -> next partition's stream 0, handled by two small
partition-shift SBUF->SBUF DMAs (A = S_0[p+1], U' = U_{0,1}[p+1]) whose
boundary row 127 is memset to zero -- which also zeroes the 3 invalid
n-gram starts 4093..4095 automatically.

Pipeline: 8 groups of 4 streams. Per group: feat DMA + signal gather ->
bind (S = sig*feat, in place over feat) -> U_s = S_s . S_{s+1}(+1) ->
Q_s = U_s . U_{s+2}(+2) -> T_t = Q_t + Q_{t+16} -> PSUM-accumulated
ones-matmul over partitions (3 segs x 16 streams).  U overwrites the dead
gathered-signal buffer, Q overwrites dead S, T overwrites dead U, so peak
SBUF is ~2 x 80KB/partition.

Index quantization is bit-exact via a host-built bucket LUT: b =
trunc/round((x-MIN)*NB/RANGE) (any rounding within +-1 bucket is fine by
construction), then idx = base[b] + (x >= t[b]) where each bucket's
extended window provably contains at most one of the 999 exact fp32
thresholds (bisected so that #{thr <= x} == reference idx).

All values are +/-1 so bf16 is exact; bundle partials are integers < 2^12
so fp32 PSUM is exact; the output sign never sees zero (4093 odd terms).
"""
import sys

sys.path.insert(0, "/opt/trn_rl_repo")

import numpy as np
import ml_dtypes

import concourse.bass as bass
import concourse.bacc as bacc
import concourse.tile as tile
import concourse.mybir as mybir
from concourse.bass_utils import run_bass_kernel_spmd
from concourse import library_config

# ---- problem constants ----
MAX_VAL = 52000.0
MIN_VAL = -53000.0
RANGE = MAX_VAL - MIN_VAL
NUM_LEVELS = 1000
NGRAM = 4
D = 10000
NFEAT = 4096
NCORE = 8

ROLL = NGRAM - 1
SLICE = D // NCORE            # 1250 output cols per core

NS = 32                       # streams (rows) per partition
TW = 1280                     # stream pitch (2560B, dma_gather 256B-multiple)
NG = 8                        # pipeline groups
GS = NS // NG                 # 4 streams per group

NB = 4096                     # quantizer buckets
NBP = NB + 4                  # padded bucket table rows
BSCALE = float(np.float32(NB / RANGE))

SEGS = [(0, 512), (512, 1024), (1024, 1252)]   # PSUM-bank matmul segs

F32 = mybir.dt.float32
BF16 = mybir.dt.bfloat16
I32 = mybir.dt.int32
I16 = mybir.dt.int16
_BF = ml_dtypes.bfloat16

NTH = NUM_LEVELS - 1


# ---------------------------------------------------------------- host prep
def _f2o(u):
    b = u.view(np.uint32).astype(np.int64)
    return np.where(b < 0x80000000, b + 0x80000000, 0xFFFFFFFF - b)


def _o2f(o):
    b = np.where(o >= 0x80000000, o - 0x80000000, 0xFFFFFFFF - o).astype(np.uint64)
    return b.astype(np.uint32).view(np.float32)


def _g(v):
    v = v.astype(np.float32)
    t = (v - np.float32(MIN_VAL)).astype(np.float32)
    t = (t / np.float32(MAX_VAL - MIN_VAL)).astype(np.float32)
    t = (t * np.float32(NUM_LEVELS - 1)).astype(np.float32)
    return np.clip(np.round(t), 0.0, float(NUM_LEVELS - 1))


def _thresholds():
    ks = np.arange(1, NUM_LEVELS, dtype=np.float32)
    lo = _f2o(np.full(ks.shape, np.float32(MIN_VAL) - np.float32(2.0)))
    hi = _f2o(np.full(ks.shape, np.float32(MAX_VAL) + np.float32(2.0)))
    for _ in range(64):
        mid = (lo + hi) // 2
        ge = _g(_o2f(mid)) >= ks
        hi = np.where(ge, mid, hi)
        lo = np.where(ge, lo, mid)
        if np.all(hi - lo <= 1):
            break
    return _o2f(hi)


def _bucket_table():
    """(NBP,) t and (NBP,) base f32 tables: idx(x) = base[b] + (x >= t[b])
    for any device bucket b within +-1.02 of (x-MIN)*NB/RANGE."""
    thr = _thresholds().astype(np.float64)          # (999,) sorted
    w = RANGE / NB
    t = np.full(NBP, 3.0e38, dtype=np.float32)
    base = np.zeros(NBP, dtype=np.float32)
    bs = np.arange(NBP, dtype=np.float64)
    lo = MIN_VAL + (bs - 1.02) * w
    hi = MIN_VAL + (bs + 1.02) * w
    for b in range(NBP):
        inb = np.nonzero((thr > lo[b]) & (thr <= hi[b]))[0]
        assert len(inb) <= 1, f"bucket {b} holds {len(inb)} thresholds"
        base[b] = np.count_nonzero(thr <= lo[b])
        if len(inb):
            t[b] = np.float32(thr[inb[0]])
    w64 = np.zeros((NBP, 64), dtype=np.float32)
    w64[:, 0] = t
    w64[:, 1] = base
    return w64


_CACHE = {}


def _host_constants():
    if "thr" not in _CACHE:
        _CACHE["thr"] = np.tile(_thresholds()[None, :], (128, 1)).astype(np.float32)
        _CACHE["zrow"] = np.zeros((1, 2 * TW), dtype=_BF)
    return _CACHE


# ---------------------------------------------------------------- program
def _build_program():
    nc = bacc.Bacc("TRN2", target_bir_lowering=False, debug=False,
                   num_devices=NCORE)

    x32_d = nc.dram_tensor("x32", (128, NS), F32, kind="ExternalInput")
    thr_d = nc.dram_tensor("thr", (128, NTH), F32, kind="ExternalInput")
    table_d = nc.dram_tensor("table", (NUM_LEVELS, TW), BF16,
                             kind="ExternalInput")
    feat_d = nc.dram_tensor("feat", (NG, 128, GS * TW), BF16,
                            kind="ExternalInput")
    zrow_d = nc.dram_tensor("zrow", (1, 2 * TW), BF16, kind="ExternalInput")
    out_d = nc.dram_tensor("out", (1, SLICE), F32, kind="ExternalOutput")
    if DEBUG:
        dbg_idx_d = nc.dram_tensor("dbg_idx", (128, NS), I32,
                                   kind="ExternalOutput")
        dbg_sig_d = nc.dram_tensor("dbg_sig", (128, TW), BF16,
                                   kind="ExternalOutput")
        dbg_s_d = nc.dram_tensor("dbg_s", (128, TW), BF16,
                                 kind="ExternalOutput")
        dbg_acc_d = nc.dram_tensor("dbg_acc", (1, 1252), F32,
                                   kind="ExternalOutput")

    # raw tensors for partition-shifted copies (row 127 stays zero)
    a_raw = nc.alloc_sbuf_tensor("a_shift", [128, TW], BF16).ap()
    up_raw = nc.alloc_sbuf_tensor("up_shift", [128, 2 * TW], BF16).ap()

    with tile.TileContext(nc) as tc:
        with tc.tile_pool(name="const", bufs=1) as cpool, \
             tc.tile_pool(name="work", bufs=1) as wpool, \
             tc.tile_pool(name="pacc", bufs=1, space="PSUM") as pacc:

            # ---- constants / index computation ----
            onr = cpool.tile([128, 1], BF16, tag="onr")
            nc.vector.memset(onr[:, :], 1.0)
            nc.scalar.dma_start(out=a_raw[127:128, :], in_=zrow_d[0:1, 0:TW])
            nc.scalar.dma_start(out=up_raw[127:128, :], in_=zrow_d[0:1, :])

            x32 = cpool.tile([128, NS], F32, tag="x32")
            nc.sync.dma_start(out=x32[:, :], in_=x32_d[:, :])
            thr = cpool.tile([128, NTH], F32, tag="thr")
            nc.sync.dma_start(out=thr[:, :], in_=thr_d[:, :])

            # idx[p, s] = #{thr <= x[p, s]} via is_le compare with fused
            # free-dim accumulate; computed one pipeline group ahead
            ge = cpool.tile([128, NTH], BF16, tag="ge")
            idxf = cpool.tile([128, NS], F32, tag="idxf")
            idxn = cpool.tile([128, NS], I32, tag="idxn")

            def idx_group(g):
                for j in range(GS):
                    s = g * GS + j
                    nc.vector.tensor_scalar(
                        out=ge[:, :], in0=thr[:, :],
                        scalar1=x32[:, s:s + 1], scalar2=0.0,
                        op0=mybir.AluOpType.is_le,
                        op1=mybir.AluOpType.add,
                        accum_out=idxf[:, s:s + 1])
                nc.vector.tensor_copy(out=idxn[:, g * GS:(g + 1) * GS],
                                      in_=idxf[:, g * GS:(g + 1) * GS])

            idx_group(0)
            if DEBUG:
                nc.sync.dma_start(out=dbg_idx_d[:, :], in_=idxn[:, :])

            # ---- main buffers ----
            sb = wpool.tile([128, NS * TW], BF16, tag="sb")    # feat -> S -> Q
            gb = wpool.tile([128, NS * TW], BF16, tag="gb")    # sig -> U -> T
            sb_r = sb[:, :].rearrange("p (s w) -> p s w", s=NS)
            gb_r = gb[:, :].rearrange("p (s w) -> p s w", s=NS)

            acc = pacc.tile([1, 1252], F32, tag="acc")

            def u_window(lo, hi):
                """U_s = S_s * S_{s+1}[+1] for s in [lo, hi) (intra-partition)."""
                nc.vector.tensor_tensor(
                    out=gb_r[:, lo:hi, 0:1254],
                    in0=sb_r[:, lo:hi, 0:1254],
                    in1=sb_r[:, lo + 1:hi + 1, 1:1255],
                    op=mybir.AluOpType.mult)

            def q_window(lo, hi):
                """Q_s = U_s * U_{s+2}[+2] for s in [lo, hi) (intra-partition)."""
                nc.vector.tensor_tensor(
                    out=sb_r[:, lo:hi, 0:1252],
                    in0=gb_r[:, lo:hi, 0:1252],
                    in1=gb_r[:, lo + 2:hi + 2, 2:1254],
                    op=mybir.AluOpType.mult)

            def q_matmuls(s):
                """accumulate Q_s (in sb) into the PSUM bundle accumulator."""
                for a0, a1 in SEGS:
                    nc.tensor.matmul(out=acc[0:1, a0:a1],
                                     lhsT=onr[:, 0:1],
                                     rhs=sb[:, s * TW + a0:s * TW + a1],
                                     start=(s == 0), stop=(s == 31))

            # ---- pipelined groups ----
            for g in range(NG):
                s0 = g * GS
                nc.sync.dma_start(out=sb[:, s0 * TW:(s0 + GS) * TW],
                                  in_=feat_d[g, :, :])
                for j in range(GS):
                    s = g * GS + j
                    nc.gpsimd.indirect_dma_start(
                        out=gb[:, s * TW:(s + 1) * TW], out_offset=None,
                        in_=table_d[:, :],
                        in_offset=bass.IndirectOffsetOnAxis(
                            ap=idxn[:, s:s + 1], axis=0),
                        element_offset=0)
                if g + 1 < NG:
                    idx_group(g + 1)
                if DEBUG and g == 0:
                    nc.sync.dma_start(out=dbg_sig_d[:, :], in_=gb[:, 0:TW])
                # bind S = sig * feat (in place over feat)
                nc.vector.tensor_tensor(
                    out=sb_r[:, s0:s0 + GS, :],
                    in0=sb_r[:, s0:s0 + GS, :],
                    in1=gb_r[:, s0:s0 + GS, :],
                    op=mybir.AluOpType.mult)
                if DEBUG and g == 0:
                    nc.sync.dma_start(out=dbg_s_d[:, :], in_=sb[:, 0:TW])

                if g == 0:
                    # A[p] = S_0[p+1] for U_31 (boundary row 127 is zero)
                    nc.scalar.dma_start(out=a_raw[0:127, :],
                                        in_=sb[1:128, 0:TW])
                    u_window(0, GS - 1)                      # U_0..2
                    # U'[p] = U_{0,1}[p+1] for Q_30,31
                    nc.scalar.dma_start(out=up_raw[0:127, :],
                                        in_=gb[1:128, 0:2 * TW])
                else:
                    u_window(s0 - 1, s0 + GS - 1)            # U_{4g-1}..{4g+2}
                    # Q streams [4(g-1) .. 4(g-1)+3] need U <= 4g+1 (done)
                    q0 = (g - 1) * GS
                    q_window(q0, q0 + GS)
                    for s in range(q0, q0 + GS):
                        q_matmuls(s)

            # ---- tail: boundary streams ----
            # U_31 = S_31 * A[+1]  (all 2D APs)
            nc.vector.tensor_tensor(
                out=gb[:, 31 * TW:31 * TW + 1254],
                in0=sb[:, 31 * TW:31 * TW + 1254],
                in1=a_raw[:, 1:1255],
                op=mybir.AluOpType.mult)
            # Q_28,29 (need U_30, U_31)
            q_window(28, 30)
            # Q_30 = U_30 * U'_0[+2];  Q_31 = U_31 * U'_1[+2]
            up_r = up_raw[:, :].rearrange("p (s w) -> p s w", s=2)
            nc.vector.tensor_tensor(
                out=sb_r[:, 30:32, 0:1252],
                in0=gb_r[:, 30:32, 0:1252],
                in1=up_r[:, 0:2, 2:1254],
                op=mybir.AluOpType.mult)
            for s in range(28, 32):
                q_matmuls(s)

            # ---- sign + output ----
            if DEBUG:
                dacc = wpool.tile([1, 1252], F32, tag="dacc")
                nc.scalar.copy(out=dacc[:, :], in_=acc[0:1, :])
                nc.sync.dma_start(out=dbg_acc_d[0:1, :], in_=dacc[:, :])
            t1 = wpool.tile([1, SLICE], F32, tag="fin2")
            nc.vector.tensor_scalar(out=t1[:, :], in0=acc[0:1, 0:SLICE],
                                    scalar1=0.0, scalar2=2.0,
                                    op0=mybir.AluOpType.is_gt,
                                    op1=mybir.AluOpType.mult)
            sg = wpool.tile([1, SLICE], F32, tag="fin3")
            nc.vector.tensor_scalar(out=sg[:, :], in0=t1[:, :], scalar1=-1.0,
                                    scalar2=None, op0=mybir.AluOpType.add)
            nc.sync.dma_start(out=out_d[0:1, :], in_=sg[:, :])

    nc.compile()
    return nc


TRACE = False
DEBUG = False
LAST_RESULT = None


def _make_in_maps(xf, sw, fw, consts):
    in_maps = []
    x32 = xf.reshape(128, NS).astype(np.float32)

    for m in range(NCORE):
        c0 = SLICE * m
        cols = (c0 + np.arange(TW)) % D
        table = sw[:, cols].astype(_BF)                       # (1000, TW)
        fwc = fw[:, cols].astype(_BF)                         # (4096, TW)
        feat = np.ascontiguousarray(
            fwc.reshape(128, NG, GS, TW)
               .transpose(1, 0, 2, 3)
               .reshape(NG, 128, GS * TW))
        in_maps.append({
            "x32": x32,
            "thr": consts["thr"],
            "table": table,
            "feat": feat,
            "zrow": consts["zrow"],
        })
    return in_maps


def kernel(x, signals_weight, feat_weight):
    global LAST_RESULT
    consts = _host_constants()

    if "nc" not in _CACHE:
        _CACHE["nc"] = _build_program()
    nc = _CACHE["nc"]

    xf = np.asarray(x, dtype=np.float32).reshape(-1)
    sw = np.asarray(signals_weight, dtype=np.float32)
    fw = np.asarray(feat_weight, dtype=np.float32)
    in_maps = _make_in_maps(xf, sw, fw, consts)

    res = run_bass_kernel_spmd(nc, in_maps, list(range(NCORE)), trace=TRACE)
    LAST_RESULT = res
    full = np.concatenate(
        [np.asarray(res.results[m]["out"], dtype=np.float32).reshape(-1)
         for m in range(NCORE)])
    return np.roll(full, ROLL)[None, :]
